# revision 8
# baseline (speedup 1.0000x reference)
"""Multi-head causal attention (B=8, T=1024, C=1024, H=16, hs=64) on 8 trn2 cores.

Data-parallel over batch: core b computes full attention for x[b].

Device algorithm (per core), all matmuls bf16 inputs / fp32 PSUM accum:
  - xT [C, T] resident in SBUF (host pre-transposed, bf16).
  - v computed for all heads up-front: v_all[s, head, s_tile, 0:64], with a
    ones column at index 64 so the AV matmul also produces softmax denominators.
  - per head-pair (2 heads packed on partitions): qT, kT = W^T @ xT -> [128, T].
  - scores transposed: scT[s_tile, t] = kT_chunk^T @ qT (only causal blocks
    s_tile <= t), exp fused on ScalarE (scale=1/8), diagonal 128x128 block
    masked by a tril multiply on VectorE.
  - out^T[65, t] accumulated over s chunks: lhsT = [v | 1], rhs = expT.
    Row 64 = sum(exp) = softmax denominator.
  - per 128-col chunk: PE-transpose [65,128] -> [128,65], reciprocal of col 64,
    normalize rows while copying PSUM->SBUF on ScalarE, DMA to out.
"""

import numpy as np
import ml_dtypes

import concourse.bass as bass
import concourse.mybir as mybir
from concourse import bacc
from concourse.tile import TileContext
from concourse.bass import ds, ts
from concourse.bass_utils import run_bass_kernel_spmd
from concourse.masks import make_identity, make_upper_triangular

BF16 = mybir.dt.bfloat16
F32 = mybir.dt.float32

B, T, C, H, HS = 8, 1024, 1024, 16, 64
P = 128
CK = C // P       # 8 contraction chunks
TT = T // P       # 8 t tiles
PAIRS = H // 2    # 8 head pairs
HALF = 512

_BUILT = None


def build_nc():
    nc = bacc.Bacc("TRN2", target_bir_lowering=False, debug=False)
    # [p, c, t] : xT[C, T] chunked; partition p, chunk c -> row 128c+p of xT
    xt = nc.dram_tensor("xt", [P, CK, T], BF16, kind="ExternalInput")
    # [proj(q,k), pair, p, c, f] : lhsT chunks, f = 2 heads x 64 stacked
    wqk = nc.dram_tensor("wqk", [2, PAIRS, P, CK, P], BF16, kind="ExternalInput")
    # [p, c, pair, f]
    wv = nc.dram_tensor("wv", [P, CK, PAIRS, P], BF16, kind="ExternalInput")
    out = nc.dram_tensor("out", [T, H * HS], F32, kind="ExternalOutput")

    with TileContext(nc) as tc:
        with (
            tc.tile_pool(name="const", bufs=1) as constp,
            tc.tile_pool(name="wpool", bufs=4) as wpool,
            tc.tile_pool(name="qkpool", bufs=4) as qkp,
            tc.tile_pool(name="exppool", bufs=20) as expp,
            tc.tile_pool(name="avspool", bufs=4) as avsp,
            tc.tile_pool(name="smallpool", bufs=12) as smallp,
            tc.tile_pool(name="psA", bufs=2, space="PSUM") as psA,
            tc.tile_pool(name="psS", bufs=3, space="PSUM") as psS,
            tc.tile_pool(name="psV", bufs=2, space="PSUM") as psV,
            tc.tile_pool(name="psT", bufs=1, space="PSUM") as psT,
        ):
            xt_sb = constp.tile([P, CK, T], BF16)
            for c in range(CK):
                nc.sync.dma_start(xt_sb[:, c, :], xt[:, c, :])
            wv_sb = constp.tile([P, CK, PAIRS, P], BF16)
            nc.sync.dma_start(wv_sb[:, :, :, :], wv[:, :, :, :])
            ident = constp.tile([P, P], F32)
            make_identity(nc, ident)
            mask = constp.tile([P, P], BF16)
            make_upper_triangular(nc, mask, val=1.0, diag=True)
            # [s_p, head, s_tile, 64 v cols + 1 ones col]
            v_all = constp.tile([P, H, TT, HS + 1], BF16)
            nc.gpsimd.memset(v_all[:, :, :, HS:HS + 1], 1.0)

            # ---- v for all heads ----
            # lhsT (= xt chunk) stays loaded across both pair-group matmuls
            for j in range(TT):
                pvs = [psA.tile([P, HALF], F32, tag="ps", name=f"pv{j}_{g}")
                       for g in range(2)]
                for c in range(CK):
                    for pg in range(2):  # pairs 4pg..4pg+3
                        nc.tensor.matmul(
                            pvs[pg][:, :],
                            xt_sb[:, c, ts(j, P)],
                            wv_sb[:, c, ds(4 * pg, 4), :],
                            start=(c == 0),
                            stop=(c == CK - 1),
                        )
                for pg in range(2):
                    # pv cols are (head0..head7 of the group) x 64 in order
                    nc.vector.tensor_copy(
                        v_all[:, ds(8 * pg, 8), j, 0:HS],
                        pvs[pg].rearrange("p (g d) -> p g d", d=HS),
                    )

            # ---- attention per head pair ----
            for pair in range(PAIRS):
                wq_sb = wpool.tile([P, CK, P], BF16, tag="w")
                nc.sync.dma_start(wq_sb[:, :, :], wqk[0, pair, :, :, :])
                wk_sb = wpool.tile([P, CK, P], BF16, tag="w")
                nc.sync.dma_start(wk_sb[:, :, :], wqk[1, pair, :, :, :])
                qT = qkp.tile([P, T], BF16, tag="qk")
                kT = qkp.tile([P, T], BF16, tag="qk")
                for wsb, dst in ((wq_sb, qT), (wk_sb, kT)):
                    # both halves in flight -> PSUM bank-alternating accumulation
                    pps = [psA.tile([P, HALF], F32, tag="ps", name=f"pp{pair}_{g}")
                           for g in range(2)]
                    for c in range(CK):
                        for g in range(2):
                            nc.tensor.matmul(
                                pps[g][:, :],
                                wsb[:, c, :],
                                xt_sb[:, c, ds(HALF * g, HALF)],
                                start=(c == 0),
                                stop=(c == CK - 1),
                            )
                    for g in range(2):
                        nc.vector.tensor_copy(dst[:, ds(HALF * g, HALF)], pps[g][:, :])

                # scores^T + exp for BOTH heads, interleaved so the two K=64
                # matmuls occupy row groups (0,0) and (64,0) concurrently.
                es = ([], [])
                for i in range(TT):
                    t0 = P * i
                    for w in range(2):
                        e = expp.tile([P, T], BF16, tag="exp", name=f"e{i}_{w}")
                        es[w].append(e)
                    spans = [(t0, HALF), (HALF, T)] if t0 < HALF else [(t0, T)]
                    for a, b in spans:
                        scs = [psS.tile([P, HALF], F32, tag="sc", name=f"sc{i}_{w}")
                               for w in range(2)]
                        for w in range(2):
                            po = HS * w
                            nc.tensor.matmul(
                                scs[w][:, 0:b - a],
                                kT[ds(po, HS), ds(t0, P)],
                                qT[ds(po, HS), ds(a, b - a)],
                            )
                        for w in range(2):
                            nc.scalar.activation(
                                es[w][i][:, ds(a, b - a)],
                                scs[w][:, 0:b - a],
                                mybir.ActivationFunctionType.Exp,
                                scale=HS ** -0.5,
                            )
                    for w in range(2):
                        nc.vector.tensor_tensor(
                            es[w][i][:, ds(t0, P)], es[w][i][:, ds(t0, P)],
                            mask[:, :], mybir.AluOpType.mult,
                        )
                # out^T = [v|1]^T @ exp^T, accumulated over s chunks;
                # heads interleaved -> bank-alternating PSUM writes
                for hh in range(2):
                    av_pair = [psV.tile([HS + 1, HALF], F32, tag="av",
                                        name=f"av{hh}_{w}") for w in range(2)]
                    contrib = [i for i in range(TT) if P * i < HALF * (hh + 1)]
                    for idx, i in enumerate(contrib):
                        g0 = max(HALF * hh, P * i)
                        g1 = HALF * (hh + 1)
                        for w in range(2):
                            nc.tensor.matmul(
                                av_pair[w][:, ds(g0 - HALF * hh, g1 - g0)],
                                v_all[:, 2 * pair + w, i, :],
                                es[w][i][:, ds(g0, g1 - g0)],
                                start=(idx == 0),
                                stop=(idx == len(contrib) - 1),
                            )
                    avs_pair = []
                    for w in range(2):
                        avs = avsp.tile([HS + 1, HALF], F32, tag="avs",
                                        name=f"avs{hh}_{w}")
                        nc.vector.tensor_copy(avs[:, :], av_pair[w][:, :])
                        avs_pair.append(avs)
                    for w in range(2):
                        h = 2 * pair + w
                        for jj in range(4):
                            j = 4 * hh + jj
                            tr = psT.tile([P, HS + 1], F32, tag="tr")
                            nc.tensor.transpose(
                                tr[:, :], avs_pair[w][:, ds(P * jj, P)],
                                ident[0:HS + 1, 0:HS + 1],
                            )
                            recip = smallp.tile([P, 1], F32, tag="recip")
                            nc.vector.reciprocal(recip[:, :], tr[:, HS:HS + 1])
                            osb = smallp.tile([P, HS], F32, tag="osb")
                            nc.vector.tensor_scalar_mul(
                                osb[:, :], tr[:, 0:HS], recip[:, :],
                            )
                            nc.sync.dma_start(out[ds(P * j, P), ds(HS * h, HS)], osb[:, :])
    nc.compile()
    return nc


def get_nc():
    global _BUILT
    if _BUILT is None:
        _BUILT = build_nc()
    return _BUILT


def prep_inputs(x, Wq, Wk, Wv):
    """Host-side shard + layout prep. Returns in_maps (one dict per core)."""
    x = np.asarray(x, dtype=np.float32)
    Wq = np.asarray(Wq, dtype=np.float32)
    Wk = np.asarray(Wk, dtype=np.float32)
    Wv = np.asarray(Wv, dtype=np.float32)
    bf = ml_dtypes.bfloat16

    # xT[b]: [C, T] -> [p, c, t] with row 128c+p
    xts = []
    for b in range(B):
        xT = np.ascontiguousarray(x[b].T)          # [C, T]
        xts.append(xT.reshape(CK, P, T).transpose(1, 0, 2).astype(bf))

    def pack_pairs(W):
        # [H, C, hs] -> [pair, C, 128] -> [pair, p, c, f]
        Wp = W.reshape(PAIRS, 2, C, HS).transpose(0, 2, 1, 3).reshape(PAIRS, C, P)
        return Wp.reshape(PAIRS, CK, P, P).transpose(0, 2, 1, 3)  # [pair, p, c, f]

    wq_p = pack_pairs(Wq)
    wk_p = pack_pairs(Wk)
    wqk_host = np.stack([wq_p, wk_p], axis=0).astype(bf)  # [2, pair, p, c, f]
    # wv: [p, c, pair, f]
    wv_host = np.ascontiguousarray(pack_pairs(Wv).transpose(1, 2, 0, 3)).astype(bf)

    return [
        {"xt": np.ascontiguousarray(xts[b]), "wqk": wqk_host, "wv": wv_host}
        for b in range(B)
    ]


def run_on_device(in_maps, **kwargs):
    nc = get_nc()
    return run_bass_kernel_spmd(nc, in_maps, list(range(B)), **kwargs)


def kernel(x, Wq, Wk, Wv):
    in_maps = prep_inputs(x, Wq, Wk, Wv)
    res = run_on_device(in_maps)
    return np.stack([res.results[b]["out"] for b in range(B)], axis=0)
